# revision 1
# baseline (speedup 1.0000x reference)
"""Trainium2 Bass kernel for a KAN layer (piecewise-cubic spline edges).

y[b, j] = scale[j] * sum_i sum_p coeff[j, i, seg(x[b,i]), p] * t(x[b,i])^p

with 9 uniform segments on [-1, 1], t the within-segment coordinate.

Strategy:
  * Recast as one-hot-masked GEMM: y^T[j, b] = sum_{s,p,ichunk}
        coeffT[s,p,ichunk,:,j]^T @ (mask_s * t^p)[ichunk,:,b]
  * 8-way data parallel over batch (each core: 512 batch cols, full OUT).
  * Masked-power tiles built on DVE/ACT/GPSIMD, matmuls in float32r
    (fp32 stored, fp22 multiply, fp32 accumulate) at full PE rate.
"""

import numpy as np

import concourse.bass as bass
import concourse.mybir as mybir
from concourse import bacc
from concourse.tile import TileContext
from concourse.bass_utils import run_bass_kernel_spmd

AF = mybir.ActivationFunctionType
OP = mybir.AluOpType
F32 = mybir.dt.float32
F32R = mybir.dt.float32r

B, IN, OUT = 4096, 512, 512
S, P = 9, 4            # segments, polynomial terms
NC = 8                 # cores
NB = B // NC           # local batch (moving free dim)
ICH = IN // 128        # input chunks (contraction tiles)
JT = OUT // 128        # output-row tiles
UMAX = float(np.nextafter(np.float32(9.0), np.float32(0.0)))

# Tunables
AT_BUFS = 6            # in-flight masked-power tile groups
CT_BUFS = 4            # in-flight coeff tile groups
MT3_ON_GPSIMD = False  # build t^3 tiles on GPSIMD (else VectorE)
DMA_BEFORE_AT = False  # emit coeff DMA before masked-power ops
JT_OUTER = True        # matmul inner loops: jt outer / p inner

LAST_EXEC_NS = None
LAST_RESULTS = None
LAST_NC = None
LAST_IN_MAPS = None


def _build_nc():
    nc = bacc.Bacc("TRN2", target_bir_lowering=False, debug=False, num_devices=NC)

    xt_d = nc.dram_tensor("xt", [IN, NB], F32, kind="ExternalInput")
    cf_d = nc.dram_tensor("coeffr", [S * ICH, 128, P * JT * 128], F32R,
                          kind="ExternalInput")
    sc_d = nc.dram_tensor("scale", [OUT, 1], F32, kind="ExternalInput")
    yt_d = nc.dram_tensor("yt", [OUT, NB], F32, kind="ExternalOutput")

    with TileContext(nc) as tc:
        with (
            tc.tile_pool(name="xp", bufs=1) as xp,
            tc.tile_pool(name="atp", bufs=AT_BUFS) as atp,
            tc.tile_pool(name="ctp", bufs=CT_BUFS) as ctp,
            tc.tile_pool(name="outp", bufs=2) as outp,
            tc.tile_pool(name="pp", bufs=1, space="PSUM") as pp,
        ):
            xt_sb = xp.tile([128, ICH, NB], F32, name="xt_sb")
            xt_r = xt_d.rearrange("(c p) b -> p c b", p=128)
            for ic in range(ICH):
                nc.sync.dma_start(xt_sb[:, ic], xt_r[:, ic])
            sc_sb = xp.tile([128, JT, 1], F32, name="sc_sb")
            nc.sync.dma_start(sc_sb, sc_d.rearrange("(c p) o -> p c o", p=128))

            u_sb = xp.tile([128, ICH, NB], F32, name="u_sb")
            t_sb = xp.tile([128, ICH, NB], F32, name="t_sb")
            seg_sb = xp.tile([128, ICH, NB], F32, name="seg_sb")
            segi_sb = xp.tile([128, ICH, NB], mybir.dt.int32, name="segi_sb")

            for ic in range(ICH):
                xs = xt_sb[:, ic]
                us = u_sb[:, ic]
                ts = t_sb[:, ic]
                ss = seg_sb[:, ic]
                # u2 = clip(x,-1,1)*4.5 + 4.0 in [-0.5, 8.5]; RNE(u2) == floor
                # of the segment coordinate (verified exact vs searchsorted).
                nc.vector.tensor_scalar(us, xs, 1.0, -1.0, OP.min, OP.max)
                nc.vector.tensor_scalar(us, us, 4.5, 4.0, OP.mult, OP.add)
                nc.vector.tensor_copy(segi_sb[:, ic], us)            # RNE -> int32
                nc.vector.tensor_copy(ss, segi_sb[:, ic])            # back to f32
                # t = (u2 + 0.5) - seg
                nc.vector.scalar_tensor_tensor(ts, us, 0.5, ss, OP.add, OP.subtract)

            ps = [pp.tile([128, NB], F32, name=f"ps{jt}", tag=f"ps{jt}")
                  for jt in range(JT)]

            for s in range(S):
                for ic in range(ICH):
                    at = atp.tile([128, P, NB], F32R, name=f"at_{s}_{ic}", tag="at")
                    ct = ctp.tile([128, JT, P * 128], F32R, name=f"ct_{s}_{ic}",
                                  tag="ct")
                    cf_g = cf_d[s * ICH + ic].rearrange("p (j q) -> p j q", j=JT)
                    if DMA_BEFORE_AT:
                        nc.sync.dma_start(ct, cf_g)
                    ts = t_sb[:, ic]
                    nc.vector.tensor_scalar(at[:, 0], seg_sb[:, ic], float(s), None,
                                            OP.is_equal)
                    nc.vector.tensor_mul(at[:, 1], at[:, 0], ts)
                    nc.scalar.activation(at[:, 2], at[:, 1], AF.Square)
                    eng3 = nc.gpsimd if MT3_ON_GPSIMD else nc.vector
                    eng3.tensor_mul(at[:, 3], at[:, 2], at[:, 1])
                    if not DMA_BEFORE_AT:
                        nc.sync.dma_start(ct, cf_g)

                    first = (s == 0 and ic == 0)
                    last = (s == S - 1 and ic == ICH - 1)
                    pjt = ([(p, jt) for jt in range(JT) for p in range(P)]
                           if JT_OUTER else
                           [(p, jt) for p in range(P) for jt in range(JT)])
                    for p, jt in pjt:
                        nc.tensor.matmul(
                            ps[jt][:, :],
                            lhsT=ct[:, jt, p * 128:(p + 1) * 128],
                            rhs=at[:, p, :],
                            start=(first and p == 0),
                            stop=(last and p == P - 1),
                        )

            for jt in range(JT):
                ot = outp.tile([128, NB], F32, name=f"ot{jt}", tag="ot")
                nc.scalar.activation(ot, ps[jt], AF.Copy, scale=sc_sb[:, jt])
                nc.sync.dma_start(yt_d[jt * 128:(jt + 1) * 128, :], ot)

    nc.compile()
    return nc


def kernel(x, coeff, scale, _trace=False):
    global LAST_EXEC_NS, LAST_RESULTS, LAST_NC, LAST_IN_MAPS
    x = np.ascontiguousarray(np.asarray(x, dtype=np.float32))
    coeff = np.ascontiguousarray(np.asarray(coeff, dtype=np.float32))
    scale = np.ascontiguousarray(np.asarray(scale, dtype=np.float32))

    # x^T shards: [IN, NB] per core
    xt = np.ascontiguousarray(x.T)
    # Round coeff to fp22 (e8m13, what the PE multiplies in) with RNE on the
    # host so the on-device f32r truncation is lossless.
    cb = coeff.view(np.uint32)
    cb = (cb + np.uint32(0x1FF) + ((cb >> np.uint32(10)) & np.uint32(1))) & \
        np.uint32(0xFFFFFC00)
    coeff = cb.view(np.float32)
    # coeff [OUT, IN, S, P] -> tiles [(s, ic), i_in, (p, jt, j_in)]
    cr = coeff.transpose(2, 3, 1, 0)                      # [S, P, IN, OUT]
    cr = cr.reshape(S, P, ICH, 128, JT, 128)              # s p ic i_in jt j_in
    cr = cr.transpose(0, 2, 3, 4, 1, 5)                   # s ic i_in jt p j_in
    cr = np.ascontiguousarray(cr.reshape(S * ICH, 128, P * JT * 128))
    sc2 = scale.reshape(OUT, 1)

    nc = _build_nc()
    in_maps = [
        {"xt": np.ascontiguousarray(xt[:, g * NB:(g + 1) * NB]),
         "coeffr": cr, "scale": sc2}
        for g in range(NC)
    ]
    res = run_bass_kernel_spmd(nc, in_maps, core_ids=list(range(NC)),
                               trace=_trace)
    LAST_RESULTS = res
    LAST_EXEC_NS = res.exec_time_ns
    LAST_NC = nc
    LAST_IN_MAPS = in_maps

    yt = np.concatenate([res.results[g]["yt"] for g in range(NC)], axis=1)
    return np.ascontiguousarray(yt.T)



# revision 7
# speedup vs baseline: 1.7024x; 1.7024x over previous
"""Trainium2 Bass kernel for a KAN layer (piecewise-cubic spline edges).

y[b, j] = scale[j] * sum_i sum_p coeff[j, i, seg(x[b,i]), p] * t(x[b,i])^p

with 9 uniform segments on [-1, 1], t the within-segment coordinate.

Strategy (fp8 DoubleRow "C2A1"):
  * Recast as one-hot-masked GEMM; 8-way data parallel over batch.
  * Coefficients split hi/lo in fp8e4m3 (x64 scaling), packed as the two
    K-rows of a DoubleRow matmul; the moving masked-power plane is fed to
    both K-rows via a stride-0 broadcast AP. Each DoubleRow matmul thus
    computes (c_hi + c_lo)^T @ a == c^T @ a at 0.5 cycles/row — 2x the
    fp32r rate — with coeff exact to ~0.06% and only the fp8 rounding of
    the masked t^p planes (~1.1e-2 rel on the output norm) as error.
  * Masked-power planes built across DVE (mask, masked-t), ACT (copy,
    square), GPSIMD (masked t^3) so no engine exceeds the PE's ~61 us.
"""

import numpy as np
import ml_dtypes

import concourse.bass as bass
import concourse.mybir as mybir
from concourse import bacc
from concourse.tile import TileContext
from concourse.bass_utils import run_bass_kernel_spmd

AF = mybir.ActivationFunctionType
OP = mybir.AluOpType
F32 = mybir.dt.float32
F8 = mybir.dt.float8e4
E4 = ml_dtypes.float8_e4m3

B, IN, OUT = 4096, 512, 512
S, P = 9, 4            # segments, polynomial terms
NC = 8                 # cores
NB = B // NC           # local batch (moving free dim)
ICH = IN // 128        # input chunks (contraction tiles)
JT = OUT // 128        # output-row tiles
CSCALE = 64.0          # coeff quantization scale (power of 2)

# Tunables
AT_BUFS = 6            # in-flight masked-power tile groups
CT_BUFS = 4            # in-flight coeff tile groups

LAST_EXEC_NS = None
LAST_RESULTS = None
LAST_NC = None
LAST_IN_MAPS = None


def _build_nc():
    nc = bacc.Bacc("TRN2", target_bir_lowering=False, debug=False, num_devices=NC)

    xt_d = nc.dram_tensor("xt", [IN, NB], F32, kind="ExternalInput")
    cf_d = nc.dram_tensor("coeff8", [S * ICH, 128, P * JT * 2 * 128], F8,
                          kind="ExternalInput")
    sc_d = nc.dram_tensor("scale", [OUT, 1], F32, kind="ExternalInput")
    yt_d = nc.dram_tensor("yt", [OUT, NB], F32, kind="ExternalOutput")

    with TileContext(nc) as tc:
        with (
            tc.tile_pool(name="xp", bufs=1) as xp,
            tc.tile_pool(name="atp", bufs=AT_BUFS) as atp,
            tc.tile_pool(name="ctp", bufs=CT_BUFS) as ctp,
            tc.tile_pool(name="outp", bufs=2) as outp,
            tc.tile_pool(name="pp", bufs=1, space="PSUM") as pp,
        ):
            xt_sb = xp.tile([128, ICH, NB], F32, name="xt_sb")
            xt_r = xt_d.rearrange("(c p) b -> p c b", p=128)
            for ic in range(ICH):
                nc.sync.dma_start(xt_sb[:, ic], xt_r[:, ic])
            sc_sb = xp.tile([128, JT, 1], F32, name="sc_sb")
            nc.sync.dma_start(sc_sb, sc_d.rearrange("(c p) o -> p c o", p=128))

            u_sb = xp.tile([128, ICH, NB], F32, name="u_sb")
            t_sb = xp.tile([128, ICH, NB], F32, name="t_sb")
            t2_sb = xp.tile([128, ICH, NB], F32, name="t2_sb")
            seg_sb = xp.tile([128, ICH, NB], F32, name="seg_sb")
            segi_sb = xp.tile([128, ICH, NB], mybir.dt.int32, name="segi_sb")

            for ic in range(ICH):
                xs = xt_sb[:, ic]
                us = u_sb[:, ic]
                ts = t_sb[:, ic]
                ss = seg_sb[:, ic]
                # u2 = clip(x,-1,1)*4.5 + 4.0 in [-0.5, 8.5]; RNE(u2) == floor
                # of the segment coordinate (verified exact vs searchsorted).
                nc.vector.tensor_scalar(us, xs, 1.0, -1.0, OP.min, OP.max)
                nc.vector.tensor_scalar(us, us, 4.5, 4.0, OP.mult, OP.add)
                nc.vector.tensor_copy(segi_sb[:, ic], us)            # RNE -> int32
                nc.vector.tensor_copy(ss, segi_sb[:, ic])            # back to f32
                # t = (u2 + 0.5) - seg
                nc.vector.scalar_tensor_tensor(ts, us, 0.5, ss, OP.add, OP.subtract)
                nc.scalar.activation(t2_sb[:, ic], ts, AF.Square)

            ps = [pp.tile([128, NB], F32, name=f"ps{jt}", tag=f"ps{jt}")
                  for jt in range(JT)]

            for s in range(S):
                for ic in range(ICH):
                    # a-planes: p=0 mask, p=1..3 masked powers of t, fp8e4m3
                    a8 = atp.tile([128, P, NB], F8, name=f"a8_{s}_{ic}", tag="a8")
                    a1f = atp.tile([128, NB], F32, name=f"a1f_{s}_{ic}", tag="a1f")
                    ct = ctp.tile([128, P, JT, 2, 128], F8, name=f"ct_{s}_{ic}",
                                  tag="ct")
                    cf_g = cf_d[s * ICH + ic].rearrange(
                        "p (q j two m) -> p q j two m", q=P, j=JT, two=2)
                    nc.sync.dma_start(ct, cf_g)

                    ts = t_sb[:, ic]
                    ss = seg_sb[:, ic]
                    nc.vector.tensor_scalar(a8[:, 0], ss, float(s), None,
                                            OP.is_equal)
                    nc.vector.scalar_tensor_tensor(a1f, ss, float(s), ts,
                                                   OP.is_equal, OP.mult)
                    nc.scalar.activation(a8[:, 1], a1f, AF.Copy)
                    nc.scalar.activation(a8[:, 2], a1f, AF.Square)
                    nc.gpsimd.tensor_mul(a8[:, 3], a1f, t2_sb[:, ic])

                    first = (s == 0 and ic == 0)
                    last = (s == S - 1 and ic == ICH - 1)
                    for p, jt in [(p, jt) for jt in range(JT) for p in range(P)]:
                        rhs = a8[:, p].unsqueeze(1).broadcast_to([128, 2, NB])
                        nc.tensor.matmul(
                            ps[jt][:, :],
                            lhsT=ct[:, p, jt],
                            rhs=rhs,
                            start=(first and p == 0),
                            stop=(last and p == P - 1),
                            perf_mode=mybir.MatmulPerfMode.DoubleRow,
                        )

            for jt in range(JT):
                ot = outp.tile([128, NB], F32, name=f"ot{jt}", tag="ot")
                nc.scalar.activation(ot, ps[jt], AF.Copy, scale=sc_sb[:, jt])
                nc.sync.dma_start(yt_d[jt * 128:(jt + 1) * 128, :], ot)

    nc.compile()
    return nc


def kernel(x, coeff, scale, _trace=False):
    global LAST_EXEC_NS, LAST_RESULTS, LAST_NC, LAST_IN_MAPS
    x = np.ascontiguousarray(np.asarray(x, dtype=np.float32))
    coeff = np.ascontiguousarray(np.asarray(coeff, dtype=np.float32))
    scale = np.ascontiguousarray(np.asarray(scale, dtype=np.float32))

    # x^T shards: [IN, NB] per core
    xt = np.ascontiguousarray(x.T)

    # coeff [OUT, IN, S, P] -> hi/lo fp8e4m3 at x64 scale, packed for
    # DoubleRow lhsT tiles [(s, ic), i_in, (p, jt, pair, j_in)]
    c64 = coeff.transpose(2, 3, 1, 0) * np.float32(CSCALE)   # [S, P, IN, OUT]
    c_hi = c64.astype(E4)
    c_lo = (c64 - c_hi.astype(np.float32)).astype(E4)
    pack = np.stack([c_hi, c_lo])                            # [2, S, P, IN, OUT]
    pack = pack.reshape(2, S, P, ICH, 128, JT, 128)          # 2 s p ic i jt j
    pack = pack.transpose(1, 3, 4, 2, 5, 0, 6)               # s ic i p jt 2 j
    cr = np.ascontiguousarray(pack.reshape(S * ICH, 128, P * JT * 2 * 128))

    sc2 = (scale / np.float32(CSCALE)).reshape(OUT, 1)

    nc = _build_nc()
    in_maps = [
        {"xt": np.ascontiguousarray(xt[:, g * NB:(g + 1) * NB]),
         "coeff8": cr, "scale": sc2}
        for g in range(NC)
    ]
    res = run_bass_kernel_spmd(nc, in_maps, core_ids=list(range(NC)),
                               trace=_trace)
    LAST_RESULTS = res
    LAST_EXEC_NS = res.exec_time_ns
    LAST_NC = nc
    LAST_IN_MAPS = in_maps

    yt = np.concatenate([res.results[g]["yt"] for g in range(NC)], axis=1)
    return np.ascontiguousarray(yt.T)


# revision 16
# speedup vs baseline: 1.9488x; 1.1447x over previous
"""Trainium2 Bass kernel for a KAN layer (piecewise-cubic spline edges).

y[b, j] = scale[j] * sum_i sum_p coeff[j, i, seg(x[b,i]), p] * t(x[b,i])^p

with 9 uniform segments on [-1, 1], t the within-segment coordinate.

Strategy (fp8e4m3 DoubleRow matmuls at 0.5 cycles/row):
  * Recast as one-hot-masked GEMM; 8-way data parallel over batch.
  * Coefficients quantized to fp8e4m3 at x64 scale with `scale` folded in.
    For p=0,1,2 the coeff is split hi/lo and the pair occupies the two
    K-rows of one DoubleRow matmul, with the moving masked-power plane
    fed to both rows via a stride-0 broadcast AP: computes exact
    (c_hi+c_lo)^T @ a at 0.5 cycles/row. For p=3 the two K-rows pair two
    single-fp8 coeff slabs of adjacent input chunks against their two a
    planes. Measured output rel err 1.44e-2 (vs 2e-2 budget).
  * Masked-power planes (fp8) built across DVE (mask, masked-t f32),
    ACT (copy, square), GPSIMD (cube = a1f*t2), each plane quantized
    exactly once from fp32 sources.
  * Chunk-pair supergroups iterate (icp outer, s inner) so chunks 2,3
    setup hides behind the first 9 groups; coeff DMA is one transfer per
    supergroup (serialized DMA is near-critical in the cost model).
"""

import numpy as np
import ml_dtypes

import concourse.bass as bass
import concourse.mybir as mybir
from concourse import bacc
from concourse.tile import TileContext
from concourse.bass_utils import run_bass_kernel_spmd

AF = mybir.ActivationFunctionType
OP = mybir.AluOpType
F32 = mybir.dt.float32
F8 = mybir.dt.float8e4
E4 = ml_dtypes.float8_e4m3
DR = mybir.MatmulPerfMode.DoubleRow

B, IN, OUT = 4096, 512, 512
S, P = 9, 4            # segments, polynomial terms
NC = 8                 # cores
NB = B // NC           # local batch (moving free dim)
ICH = IN // 128        # input chunks (contraction tiles)
JT = OUT // 128        # output-row tiles
NSG = ICH // 2         # chunk-pair supergroups per segment
CSCALE = 64.0          # coeff quantization scale (power of 2)
# free bytes per partition of one supergroup coeff tile:
#   2 chunks * (3 hi/lo pairs * 2 + 1 single) * JT * 128
CT_FREE = 2 * 7 * JT * 128

AT_BUFS = 4            # in-flight masked-power supergroups
CT_BUFS = 3            # in-flight coeff supergroups
DEBUG_GROUPS = None    # optional [(icp, s), ...] to restrict accumulation

LAST_EXEC_NS = None
LAST_RESULTS = None
LAST_NC = None
LAST_IN_MAPS = None


def _build_nc():
    nc = bacc.Bacc("TRN2", target_bir_lowering=False, debug=False, num_devices=NC)

    xt_d = nc.dram_tensor("xt", [IN, NB], F32, kind="ExternalInput")
    cf_d = nc.dram_tensor("coeff8", [S * NSG, 128, CT_FREE], F8,
                          kind="ExternalInput")
    yt_d = nc.dram_tensor("yt", [OUT, NB], F32, kind="ExternalOutput")

    with TileContext(nc) as tc:
        with (
            tc.tile_pool(name="xp", bufs=1) as xp,
            tc.tile_pool(name="atp", bufs=AT_BUFS) as atp,
            tc.tile_pool(name="ctp", bufs=CT_BUFS) as ctp,
            tc.tile_pool(name="outp", bufs=1) as outp,
            tc.tile_pool(name="pp", bufs=1, space="PSUM") as pp,
        ):
            xt_sb = xp.tile([128, ICH, NB], F32, name="xt_sb")
            u_sb = xp.tile([128, ICH, NB], F32, name="u_sb")
            t_sb = xp.tile([128, ICH, NB], F32, name="t_sb")
            t2_sb = xp.tile([128, ICH, NB], F32, name="t2_sb")
            seg_sb = xp.tile([128, ICH, NB], F32, name="seg_sb")
            segi_sb = xp.tile([128, ICH, NB], mybir.dt.int32, name="segi_sb")
            xt_r = xt_d.rearrange("(c p) b -> p c b", p=128)

            def setup_chunk(ic):
                xs = xt_sb[:, ic]
                us = u_sb[:, ic]
                ts = t_sb[:, ic]
                ss = seg_sb[:, ic]
                # u2 = clip(x,-1,1)*4.5 + 4.0 in [-0.5, 8.5]; RNE(u2) == floor
                # of the segment coordinate (verified exact vs searchsorted).
                nc.vector.tensor_scalar(us, xs, 1.0, -1.0, OP.min, OP.max)
                nc.vector.tensor_scalar(us, us, 4.5, 4.0, OP.mult, OP.add)
                nc.vector.tensor_copy(segi_sb[:, ic], us)            # RNE -> int32
                nc.vector.tensor_copy(ss, segi_sb[:, ic])            # back to f32
                # t = (u2 + 0.5) - seg
                nc.vector.scalar_tensor_tensor(ts, us, 0.5, ss, OP.add, OP.subtract)
                nc.scalar.activation(t2_sb[:, ic], ts, AF.Square)

            # startup: chunks 0,1 first; 2,3 land behind the first 9 groups
            nc.sync.dma_start(xt_sb[:, 0], xt_r[:, 0])
            nc.sync.dma_start(xt_sb[:, 1], xt_r[:, 1])
            setup_chunk(0)
            setup_chunk(1)

            ps = pp.tile([128, JT, NB], F32, name="ps", tag="ps")
            ot = outp.tile([128, JT, NB], F32, name="ot")

            groups = [(icp, s) for icp in range(NSG) for s in range(S)]
            if DEBUG_GROUPS is not None:
                groups = [g for g in groups if g in DEBUG_GROUPS]

            first = True
            emitted23 = False
            for icp, s in groups:
                if icp == 1 and not emitted23:
                    emitted23 = True
                    nc.sync.dma_start(xt_sb[:, 2], xt_r[:, 2])
                    nc.sync.dma_start(xt_sb[:, 3], xt_r[:, 3])
                    setup_chunk(2)
                    setup_chunk(3)
                if True:
                    # coeff supergroup tile: [k, chunk, plane(7), jt, 128]
                    # plane layout per chunk: (h0,l0,h1,l1,h2,l2,c3)
                    ct = ctp.tile([128, 2, 7, JT, 128], F8,
                                  name=f"ct_{icp}_{s}", tag="ct")
                    nc.sync.dma_start(
                        ct, cf_d[icp * S + s].rearrange(
                            "p (c q j m) -> p c q j m", c=2, q=7, j=JT))

                    a8 = atp.tile([128, 2, P, NB], F8, name=f"a8_{icp}_{s}",
                                  tag="a8")
                    a1f = atp.tile([128, 2, NB], F32, name=f"a1f_{icp}_{s}",
                                   tag="a1f")
                    for k in range(2):
                        ic = 2 * icp + k
                        ss = seg_sb[:, ic]
                        nc.vector.tensor_scalar(a8[:, k, 0], ss, float(s),
                                                None, OP.is_equal)
                        nc.vector.scalar_tensor_tensor(a1f[:, k], ss, float(s),
                                                       t_sb[:, ic],
                                                       OP.is_equal, OP.mult)
                        nc.scalar.activation(a8[:, k, 1], a1f[:, k], AF.Copy)
                        nc.scalar.activation(a8[:, k, 2], a1f[:, k], AF.Square)
                        nc.gpsimd.tensor_mul(a8[:, k, 3], a1f[:, k],
                                             t2_sb[:, ic])

                    last = ((icp, s) == groups[-1])
                    for jt in range(JT):
                        for k in range(2):
                            for q in range(3):   # p=0,1,2 hi/lo pairs
                                rhs = a8[:, k, q].unsqueeze(1).broadcast_to(
                                    [128, 2, NB])
                                nc.tensor.matmul(
                                    ps[:, jt],
                                    lhsT=ct[:, k, 2 * q:2 * q + 2, jt],
                                    rhs=rhs,
                                    start=(first and k == 0 and q == 0),
                                    stop=False,
                                    perf_mode=DR,
                                )
                        # p=3: pair the two chunks' single-fp8 slabs
                        nc.tensor.matmul(
                            ps[:, jt],
                            lhsT=ct[:, :, 6, jt],
                            rhs=a8[:, :, 3],
                            start=False,
                            stop=last,
                            perf_mode=DR,
                        )
                    first = False

            inv = 1.0 / CSCALE
            nc.scalar.activation(ot[:, 1], ps[:, 1], AF.Copy, scale=inv)
            nc.scalar.activation(ot[:, 2], ps[:, 2], AF.Copy, scale=inv)
            nc.vector.tensor_scalar(ot[:, 0], ps[:, 0], inv, None, OP.mult)
            nc.vector.tensor_scalar(ot[:, 3], ps[:, 3], inv, None, OP.mult)
            nc.sync.dma_start(
                yt_d.rearrange("(j p) b -> p j b", p=128), ot)

    nc.compile()
    return nc


def kernel(x, coeff, scale, _trace=False):
    global LAST_EXEC_NS, LAST_RESULTS, LAST_NC, LAST_IN_MAPS
    x = np.ascontiguousarray(np.asarray(x, dtype=np.float32))
    coeff = np.ascontiguousarray(np.asarray(coeff, dtype=np.float32))
    scale = np.ascontiguousarray(np.asarray(scale, dtype=np.float32))

    # x^T shards: [IN, NB] per core
    xt = np.ascontiguousarray(x.T)

    # coeff [OUT, IN, S, P] * scale -> fp8 planes, packed per supergroup as
    # [s*NSG+icp, i_in, (chunk, plane7, jt, j_in)] with plane order
    # (h0,l0,h1,l1,h2,l2,c3)
    csc = coeff.transpose(2, 3, 1, 0) * (scale[None, None, None, :]
                                         * np.float32(CSCALE))  # [S,P,IN,OUT]
    c_hi = csc.astype(E4)
    c_lo = (csc - c_hi.astype(np.float32)).astype(E4)
    planes = np.empty((S, 7, IN, OUT), dtype=E4)
    for q in range(3):
        planes[:, 2 * q] = c_hi[:, q]
        planes[:, 2 * q + 1] = c_lo[:, q]
    planes[:, 6] = c_hi[:, 3]
    # [S, 7, (ich, i), (jt, j)] -> [S, ich(2*icp+k), i, 7, jt, j]
    pl = planes.reshape(S, 7, ICH, 128, JT, 128)
    pl = pl.transpose(0, 2, 3, 1, 4, 5)          # S ich i 7 jt j
    pl = pl.reshape(S, NSG, 2, 128, 7, JT, 128)  # S icp k i 7 jt j
    pl = pl.transpose(1, 0, 3, 2, 4, 5, 6)       # icp S i k 7 jt j
    cr = np.ascontiguousarray(pl.reshape(S * NSG, 128, CT_FREE))

    nc = _build_nc()
    in_maps = [
        {"xt": np.ascontiguousarray(xt[:, g * NB:(g + 1) * NB]),
         "coeff8": cr}
        for g in range(NC)
    ]
    res = run_bass_kernel_spmd(nc, in_maps, core_ids=list(range(NC)),
                               trace=_trace)
    LAST_RESULTS = res
    LAST_EXEC_NS = res.exec_time_ns
    LAST_NC = nc
    LAST_IN_MAPS = in_maps

    yt = np.concatenate([res.results[g]["yt"] for g in range(NC)], axis=1)
    return np.ascontiguousarray(yt.T)


# revision 27
# speedup vs baseline: 2.3439x; 1.2027x over previous
"""Trainium2 Bass kernel for a KAN layer (piecewise-cubic spline edges).

y[b, j] = scale[j] * sum_i sum_p coeff[j, i, seg(x[b,i]), p] * t(x[b,i])^p

with 9 uniform segments on [-1, 1], t the within-segment coordinate.

Strategy (fp8e4m3 DoubleRow matmuls at 0.5 cycles/row):
  * Recast as one-hot-masked GEMM; 8-way data parallel over batch.
  * Coefficients quantized to fp8e4m3 at x64 scale with `scale` folded in.
    For p=0,1 the coeff is split hi/lo and the pair occupies the two
    K-rows of one DoubleRow matmul, with the moving masked-power plane
    fed to both rows via a stride-0 broadcast AP: computes exact
    (c_hi+c_lo)^T @ a at 0.5 cycles/row. For p=2,3 the two K-rows pair
    the two single-fp8 coeff slabs against the (a2, a3) plane pair.
    Measured output rel err 1.75e-2 (vs 2e-2 budget), deterministic.
  * Masked-power planes (fp8) built across DVE (mask, masked-t f32,
    copy), ACT (copy, square), GPSIMD (cube = a1f*t2), each plane
    quantized exactly once from fp32 sources.
  * Chunk-pair supergroups iterate (icp outer, s inner) so chunks 2,3
    setup hides behind the first 9 groups; coeff DMA is one transfer per
    supergroup (serialized DMA is near-critical in the cost model).
"""

import numpy as np
import ml_dtypes

import concourse.bass as bass
import concourse.mybir as mybir
from concourse import bacc
from concourse.tile import TileContext
from concourse.bass_utils import run_bass_kernel_spmd

AF = mybir.ActivationFunctionType
OP = mybir.AluOpType
F32 = mybir.dt.float32
F8 = mybir.dt.float8e4
E4 = ml_dtypes.float8_e4m3
DR = mybir.MatmulPerfMode.DoubleRow

B, IN, OUT = 4096, 512, 512
S, P = 9, 4            # segments, polynomial terms
NC = 8                 # cores
NB = B // NC           # local batch (moving free dim)
ICH = IN // 128        # input chunks (contraction tiles)
JT = OUT // 128        # output-row tiles
NSG = ICH // 2         # chunk-pair supergroups per segment
CSCALE = 64.0          # coeff quantization scale (power of 2)
# free bytes per partition of one supergroup coeff tile:
#   2 chunks * (2 hi/lo pairs * 2 + 2 singles) * JT * 128
CT_FREE = 2 * 6 * JT * 128

AT_BUFS = 4            # in-flight masked-power supergroups
CT_BUFS = 3            # in-flight coeff supergroups
N_WARM = 0            # dummy PE matmuls to ride out the p-state ramp
DEBUG_GROUPS = None    # optional [(icp, s), ...] to restrict accumulation

LAST_EXEC_NS = None
LAST_RESULTS = None
LAST_NC = None
LAST_IN_MAPS = None


def _build_nc():
    nc = bacc.Bacc("TRN2", target_bir_lowering=False, debug=False, num_devices=NC)

    xt_d = nc.dram_tensor("xt", [IN, NB], F32, kind="ExternalInput")
    cf_d = nc.dram_tensor("coeff8", [S * NSG, 128, CT_FREE], F8,
                          kind="ExternalInput")
    yt_d = nc.dram_tensor("yt", [OUT, NB], F32, kind="ExternalOutput")

    with TileContext(nc) as tc:
        with (
            tc.tile_pool(name="xp", bufs=1) as xp,
            tc.tile_pool(name="atp", bufs=AT_BUFS) as atp,
            tc.tile_pool(name="ctp", bufs=CT_BUFS) as ctp,
            tc.tile_pool(name="outp", bufs=1) as outp,
            tc.tile_pool(name="pp", bufs=1, space="PSUM") as pp,
        ):
            xt_sb = xp.tile([128, ICH, NB], F32, name="xt_sb")
            u_sb = xp.tile([128, ICH, NB], F32, name="u_sb")
            t_sb = xp.tile([128, ICH, NB], F32, name="t_sb")
            t2_sb = xp.tile([128, ICH, NB], F32, name="t2_sb")
            seg_sb = xp.tile([128, ICH, NB], F32, name="seg_sb")
            segi_sb = xp.tile([128, ICH, NB], mybir.dt.int32, name="segi_sb")
            xt_r = xt_d.rearrange("(c p) b -> p c b", p=128)

            def setup_seg(ic):
                xs = xt_sb[:, ic]
                us = u_sb[:, ic]
                ss = seg_sb[:, ic]
                # u2 = clip(x,-1,1)*4.5 + 4.0 in [-0.5, 8.5]; RNE(u2) == floor
                # of the segment coordinate (verified exact vs searchsorted).
                nc.vector.tensor_scalar(us, xs, 1.0, -1.0, OP.min, OP.max)
                nc.vector.tensor_scalar(us, us, 4.5, 4.0, OP.mult, OP.add)
                nc.vector.tensor_copy(segi_sb[:, ic], us)            # RNE -> int32
                nc.vector.tensor_copy(ss, segi_sb[:, ic])            # back to f32

            def setup_t(ic):
                ts = t_sb[:, ic]
                # t = (u2 + 0.5) - seg
                nc.vector.scalar_tensor_tensor(ts, u_sb[:, ic], 0.5,
                                               seg_sb[:, ic], OP.add,
                                               OP.subtract)
                nc.scalar.activation(t2_sb[:, ic], ts, AF.Square)

            # startup: chunks 0,1 DMA first; setups emitted lazily per chunk
            # so the first group's chunk-0 planes build before chunk 1 setup
            nc.sync.dma_start(xt_sb[:, 0:2], xt_r[:, 0:2])
            setup_done = set()
            t_done = set()

            ps = [pp.tile([128, NB], F32, name=f"ps{jt}", tag=f"ps{jt}")
                  for jt in range(JT)]
            ot = [outp.tile([128, NB], F32, name=f"ot{jt}", tag=f"ot{jt}")
                  for jt in range(JT)]

            # keep the PE continuously busy through its p-state ramp window
            # so the real matmuls all run at full clock
            if N_WARM:
                wz = xp.tile([128, 2, NB], F8, name="warm_z")
                wp = pp.tile([128, NB], F32, name="warm_ps", tag="warm")
                nc.vector.memset(wz, 0.0)
                for w in range(N_WARM):
                    nc.tensor.matmul(wp, lhsT=wz[:, :, 0:128], rhs=wz,
                                     start=True, stop=True, perf_mode=DR)

            groups = [(icp, s) for icp in range(NSG) for s in range(S)]
            if DEBUG_GROUPS is not None:
                groups = [g for g in groups if g in DEBUG_GROUPS]

            first = True
            for icp, s in groups:
                if icp == 1 and 2 not in setup_done:
                    nc.sync.dma_start(xt_sb[:, 2:4], xt_r[:, 2:4])
                # coeff supergroup tile: [k, chunk, plane(6), jt, 128]
                # plane layout per chunk: (h0,l0,h1,l1,c2,c3)
                ct = ctp.tile([128, 2, 6, JT, 128], F8,
                              name=f"ct_{icp}_{s}", tag="ct")
                nc.sync.dma_start(
                    ct, cf_d[icp * S + s].rearrange(
                        "p (c q j m) -> p c q j m", c=2, q=6, j=JT))

                a8 = atp.tile([128, 2, P, NB], F8, name=f"a8_{icp}_{s}",
                              tag="a8")
                a1f = atp.tile([128, 2, NB], F32, name=f"a1f_{icp}_{s}",
                               tag="a1f")
                last = ((icp, s) == groups[-1])
                for k in range(2):
                    ic = 2 * icp + k
                    if ic not in setup_done:
                        setup_done.add(ic)
                        setup_seg(ic)
                    ss = seg_sb[:, ic]
                    nc.vector.tensor_scalar(a8[:, k, 0], ss, float(s),
                                            None, OP.is_equal)
                    if ic not in t_done:
                        t_done.add(ic)
                        setup_t(ic)
                    nc.vector.scalar_tensor_tensor(a1f[:, k], ss, float(s),
                                                   t_sb[:, ic],
                                                   OP.is_equal, OP.mult)
                    if k % 2:
                        nc.scalar.activation(a8[:, k, 1], a1f[:, k], AF.Copy)
                    else:
                        nc.vector.tensor_copy(a8[:, k, 1], a1f[:, k])
                    nc.scalar.activation(a8[:, k, 2], a1f[:, k], AF.Square)
                    nc.gpsimd.tensor_mul(a8[:, k, 3], a1f[:, k],
                                         t2_sb[:, ic])
                    if not last:
                        for jt in range(JT):
                            for q in range(3):
                                # q0/q1: (hi,lo) pair x broadcast plane;
                                # q2: (c2,c3) pair x (a2,a3) planes
                                if q < 2:
                                    rhs = a8[:, k, q].unsqueeze(1).broadcast_to(
                                        [128, 2, NB])
                                else:
                                    rhs = a8[:, k, 2:4]
                                nc.tensor.matmul(
                                    ps[jt],
                                    lhsT=ct[:, k, 2 * q:2 * q + 2, jt],
                                    rhs=rhs,
                                    start=(first and k == 0 and q == 0),
                                    stop=False,
                                    perf_mode=DR,
                                )
                if last:
                    # final group: jt-major so each bank drains while later
                    # banks still accumulate
                    inv = 1.0 / CSCALE
                    for jt in range(JT):
                        for k in range(2):
                            for q in range(3):
                                if q < 2:
                                    rhs = a8[:, k, q].unsqueeze(1).broadcast_to(
                                        [128, 2, NB])
                                else:
                                    rhs = a8[:, k, 2:4]
                                nc.tensor.matmul(
                                    ps[jt],
                                    lhsT=ct[:, k, 2 * q:2 * q + 2, jt],
                                    rhs=rhs, start=False,
                                    stop=(k == 1 and q == 2),
                                    perf_mode=DR,
                                )
                        if jt % 2:
                            nc.scalar.activation(ot[jt], ps[jt],
                                                 AF.Copy, scale=inv)
                        else:
                            nc.vector.tensor_scalar(ot[jt], ps[jt],
                                                    inv, None, OP.mult)
                        nc.sync.dma_start(
                            yt_d[jt * 128:(jt + 1) * 128, :], ot[jt])
                first = False

    nc.compile()
    return nc


def kernel(x, coeff, scale, _trace=False):
    global LAST_EXEC_NS, LAST_RESULTS, LAST_NC, LAST_IN_MAPS
    x = np.ascontiguousarray(np.asarray(x, dtype=np.float32))
    coeff = np.ascontiguousarray(np.asarray(coeff, dtype=np.float32))
    scale = np.ascontiguousarray(np.asarray(scale, dtype=np.float32))

    # x^T shards: [IN, NB] per core
    xt = np.ascontiguousarray(x.T)

    # coeff [OUT, IN, S, P] * scale -> fp8 planes, packed per supergroup as
    # [s*NSG+icp, i_in, (chunk, plane7, jt, j_in)] with plane order
    # (h0,l0,h1,l1,h2,l2,c3)
    csc = coeff.transpose(2, 3, 1, 0) * (scale[None, None, None, :]
                                         * np.float32(CSCALE))  # [S,P,IN,OUT]
    c_hi = csc.astype(E4)
    c_lo = (csc - c_hi.astype(np.float32)).astype(E4)
    planes = np.empty((S, 6, IN, OUT), dtype=E4)
    for q in range(2):
        planes[:, 2 * q] = c_hi[:, q]
        planes[:, 2 * q + 1] = c_lo[:, q]
    planes[:, 4] = c_hi[:, 2]
    planes[:, 5] = c_hi[:, 3]
    # [S, 6, (ich, i), (jt, j)] -> [S, ich(2*icp+k), i, 6, jt, j]
    pl = planes.reshape(S, 6, ICH, 128, JT, 128)
    pl = pl.transpose(0, 2, 3, 1, 4, 5)          # S ich i 6 jt j
    pl = pl.reshape(S, NSG, 2, 128, 6, JT, 128)  # S icp k i 6 jt j
    pl = pl.transpose(1, 0, 3, 2, 4, 5, 6)       # icp S i k 6 jt j
    cr = np.ascontiguousarray(pl.reshape(S * NSG, 128, CT_FREE))

    nc = _build_nc()
    in_maps = [
        {"xt": np.ascontiguousarray(xt[:, g * NB:(g + 1) * NB]),
         "coeff8": cr}
        for g in range(NC)
    ]
    res = run_bass_kernel_spmd(nc, in_maps, core_ids=list(range(NC)),
                               trace=_trace)
    LAST_RESULTS = res
    LAST_EXEC_NS = res.exec_time_ns
    LAST_NC = nc
    LAST_IN_MAPS = in_maps

    yt = np.concatenate([res.results[g]["yt"] for g in range(NC)], axis=1)
    return np.ascontiguousarray(yt.T)


# revision 30
# speedup vs baseline: 2.3454x; 1.0006x over previous
"""Trainium2 Bass kernel for a KAN layer (piecewise-cubic spline edges).

y[b, j] = scale[j] * sum_i sum_p coeff[j, i, seg(x[b,i]), p] * t(x[b,i])^p

with 9 uniform segments on [-1, 1], t the within-segment coordinate.

Strategy (fp8e4m3 DoubleRow matmuls at 0.5 cycles/row):
  * Recast as one-hot-masked GEMM; 8-way data parallel over batch.
  * Coefficients quantized to fp8e4m3 at x64 scale with `scale` folded in.
    For p=0,1 the coeff is split hi/lo and the pair occupies the two
    K-rows of one DoubleRow matmul, with the moving masked-power plane
    fed to both rows via a stride-0 broadcast AP: computes exact
    (c_hi+c_lo)^T @ a at 0.5 cycles/row. For p=2,3 the two K-rows pair
    the two single-fp8 coeff slabs against the (a2, a3) plane pair.
    Measured output rel err 1.75e-2 (vs 2e-2 budget), deterministic.
  * Masked-power planes (fp8) built across DVE (mask, masked-t f32,
    copy), ACT (copy, square), GPSIMD (cube = a1f*t2), each plane
    quantized exactly once from fp32 sources.
  * Chunk-pair supergroups iterate (icp outer, s inner) so chunks 2,3
    setup hides behind the first 9 groups; coeff DMA is one transfer per
    supergroup (serialized DMA is near-critical in the cost model).
"""

import numpy as np
import ml_dtypes

import concourse.bass as bass
import concourse.mybir as mybir
from concourse import bacc
from concourse.tile import TileContext
from concourse.bass_utils import run_bass_kernel_spmd

AF = mybir.ActivationFunctionType
OP = mybir.AluOpType
F32 = mybir.dt.float32
F8 = mybir.dt.float8e4
E4 = ml_dtypes.float8_e4m3
DR = mybir.MatmulPerfMode.DoubleRow

B, IN, OUT = 4096, 512, 512
S, P = 9, 4            # segments, polynomial terms
NC = 8                 # cores
NB = B // NC           # local batch (moving free dim)
ICH = IN // 128        # input chunks (contraction tiles)
JT = OUT // 128        # output-row tiles
NSG = ICH // 2         # chunk-pair supergroups per segment
CSCALE = 64.0          # coeff quantization scale (power of 2)
# free bytes per partition of one supergroup coeff tile:
#   2 chunks * (2 hi/lo pairs * 2 + 2 singles) * JT * 128
CT_FREE = 2 * 6 * JT * 128

AT_BUFS = 4            # in-flight masked-power supergroups
CT_BUFS = 3            # in-flight coeff supergroups
N_WARM = 0            # dummy PE matmuls to ride out the p-state ramp
DEBUG_GROUPS = None    # optional [(icp, s), ...] to restrict accumulation

LAST_EXEC_NS = None
LAST_RESULTS = None
LAST_NC = None
LAST_IN_MAPS = None


def _build_nc():
    nc = bacc.Bacc("TRN2", target_bir_lowering=False, debug=False, num_devices=NC)

    xt_d = nc.dram_tensor("xt", [IN, NB], F32, kind="ExternalInput")
    cf_d = nc.dram_tensor("coeff8", [S * NSG, 128, CT_FREE], F8,
                          kind="ExternalInput")
    yt_d = nc.dram_tensor("yt", [OUT, NB], F32, kind="ExternalOutput")

    with TileContext(nc) as tc:
        with (
            tc.tile_pool(name="xp", bufs=1) as xp,
            tc.tile_pool(name="atp", bufs=AT_BUFS) as atp,
            tc.tile_pool(name="ctp", bufs=CT_BUFS) as ctp,
            tc.tile_pool(name="outp", bufs=1) as outp,
            tc.tile_pool(name="pp", bufs=1, space="PSUM") as pp,
        ):
            xt_sb = xp.tile([128, ICH, NB], F32, name="xt_sb")
            u_sb = xp.tile([128, ICH, NB], F32, name="u_sb")
            t_sb = xp.tile([128, ICH, NB], F32, name="t_sb")
            t2_sb = xp.tile([128, ICH, NB], F32, name="t2_sb")
            seg_sb = xp.tile([128, ICH, NB], F32, name="seg_sb")
            segi_sb = xp.tile([128, ICH, NB], mybir.dt.int32, name="segi_sb")
            xt_r = xt_d.rearrange("(c p) b -> p c b", p=128)

            def setup_seg(ic):
                xs = xt_sb[:, ic]
                us = u_sb[:, ic]
                ss = seg_sb[:, ic]
                # u2 = clip(x,-1,1)*4.5 + 4.0 in [-0.5, 8.5]; RNE(u2) == floor
                # of the segment coordinate (verified exact vs searchsorted).
                nc.vector.tensor_scalar(us, xs, 1.0, -1.0, OP.min, OP.max)
                nc.vector.tensor_scalar(us, us, 4.5, 4.0, OP.mult, OP.add)
                nc.vector.tensor_copy(segi_sb[:, ic], us)            # RNE -> int32
                nc.vector.tensor_copy(ss, segi_sb[:, ic])            # back to f32

            def setup_t(ic):
                ts = t_sb[:, ic]
                # t = (u2 + 0.5) - seg
                nc.vector.scalar_tensor_tensor(ts, u_sb[:, ic], 0.5,
                                               seg_sb[:, ic], OP.add,
                                               OP.subtract)
                nc.scalar.activation(t2_sb[:, ic], ts, AF.Square)

            # startup: chunks 0,1 DMA first; setups emitted lazily per chunk
            # so the first group's chunk-0 planes build before chunk 1 setup
            nc.sync.dma_start(xt_sb[:, 0:2], xt_r[:, 0:2])
            setup_done = set()
            t_done = set()

            ps = [pp.tile([128, NB], F32, name=f"ps{jt}", tag=f"ps{jt}")
                  for jt in range(JT)]
            ot = [outp.tile([128, NB], F32, name=f"ot{jt}", tag=f"ot{jt}")
                  for jt in range(JT)]

            # keep the PE continuously busy through its p-state ramp window
            # so the real matmuls all run at full clock
            if N_WARM:
                wz = xp.tile([128, 2, NB], F8, name="warm_z")
                wp = pp.tile([128, NB], F32, name="warm_ps", tag="warm")
                nc.vector.memset(wz, 0.0)
                for w in range(N_WARM):
                    nc.tensor.matmul(wp, lhsT=wz[:, :, 0:128], rhs=wz,
                                     start=True, stop=True, perf_mode=DR)

            groups = [(icp, s) for icp in range(NSG) for s in range(S)]
            if DEBUG_GROUPS is not None:
                groups = [g for g in groups if g in DEBUG_GROUPS]

            def emit_group_tiles(icp, s):
                # coeff supergroup tile: [k, chunk, plane(6), jt, 128]
                # plane layout per chunk: (h0,l0,h1,l1,c2,c3)
                ct = ctp.tile([128, 2, 6, JT, 128], F8,
                              name=f"ct_{icp}_{s}", tag="ct")
                nc.sync.dma_start(
                    ct, cf_d[icp * S + s].rearrange(
                        "p (c q j m) -> p c q j m", c=2, q=6, j=JT))
                a8 = atp.tile([128, 2, P, NB], F8, name=f"a8_{icp}_{s}",
                              tag="a8")
                a1f = atp.tile([128, 2, NB], F32, name=f"a1f_{icp}_{s}",
                               tag="a1f")
                return ct, a8, a1f

            def emit_builders(icp, s, k, a8, a1f):
                ic = 2 * icp + k
                if ic not in setup_done:
                    setup_done.add(ic)
                    setup_seg(ic)
                ss = seg_sb[:, ic]
                nc.vector.tensor_scalar(a8[:, k, 0], ss, float(s),
                                        None, OP.is_equal)
                if ic not in t_done:
                    t_done.add(ic)
                    setup_t(ic)
                nc.vector.scalar_tensor_tensor(a1f[:, k], ss, float(s),
                                               t_sb[:, ic],
                                               OP.is_equal, OP.mult)
                if k % 2:
                    nc.scalar.activation(a8[:, k, 1], a1f[:, k], AF.Copy)
                else:
                    nc.vector.tensor_copy(a8[:, k, 1], a1f[:, k])
                nc.scalar.activation(a8[:, k, 2], a1f[:, k], AF.Square)
                nc.gpsimd.tensor_mul(a8[:, k, 3], a1f[:, k], t2_sb[:, ic])

            def mm(jt, ct, a8, k, q, start=False, stop=False):
                # q0/q1: (hi,lo) pair x broadcast plane;
                # q2: (c2,c3) pair x (a2,a3) planes
                if q < 2:
                    rhs = a8[:, k, q].unsqueeze(1).broadcast_to([128, 2, NB])
                else:
                    rhs = a8[:, k, 2:4]
                nc.tensor.matmul(ps[jt], lhsT=ct[:, k, 2 * q:2 * q + 2, jt],
                                 rhs=rhs, start=start, stop=stop,
                                 perf_mode=DR)

            first = True
            for icp, s in groups[:-2]:
                if icp == 1 and 2 not in setup_done:
                    nc.sync.dma_start(xt_sb[:, 2:4], xt_r[:, 2:4])
                ct, a8, a1f = emit_group_tiles(icp, s)
                for k in range(2):
                    emit_builders(icp, s, k, a8, a1f)
                    for jt in range(JT):
                        for q in range(3):
                            mm(jt, ct, a8, k, q,
                               start=(first and k == 0 and q == 0))
                first = False

            # final two groups: jt-major across both so each PSUM bank
            # stops and drains while later banks still accumulate
            tiles = []
            for icp, s in groups[-2:]:
                ct, a8, a1f = emit_group_tiles(icp, s)
                for k in range(2):
                    emit_builders(icp, s, k, a8, a1f)
                tiles.append((ct, a8))
            inv = 1.0 / CSCALE
            for jt in range(JT):
                for gi, (ct, a8) in enumerate(tiles):
                    for k in range(2):
                        for q in range(3):
                            mm(jt, ct, a8, k, q,
                               stop=(gi == 1 and k == 1 and q == 2))
                if jt % 2:
                    nc.scalar.activation(ot[jt], ps[jt], AF.Copy, scale=inv)
                else:
                    nc.vector.tensor_scalar(ot[jt], ps[jt], inv, None,
                                            OP.mult)
                nc.sync.dma_start(yt_d[jt * 128:(jt + 1) * 128, :], ot[jt])

    nc.compile()
    return nc


def kernel(x, coeff, scale, _trace=False):
    global LAST_EXEC_NS, LAST_RESULTS, LAST_NC, LAST_IN_MAPS
    x = np.ascontiguousarray(np.asarray(x, dtype=np.float32))
    coeff = np.ascontiguousarray(np.asarray(coeff, dtype=np.float32))
    scale = np.ascontiguousarray(np.asarray(scale, dtype=np.float32))

    # x^T shards: [IN, NB] per core
    xt = np.ascontiguousarray(x.T)

    # coeff [OUT, IN, S, P] * scale -> fp8 planes, packed per supergroup as
    # [s*NSG+icp, i_in, (chunk, plane7, jt, j_in)] with plane order
    # (h0,l0,h1,l1,h2,l2,c3)
    csc = coeff.transpose(2, 3, 1, 0) * (scale[None, None, None, :]
                                         * np.float32(CSCALE))  # [S,P,IN,OUT]
    c_hi = csc.astype(E4)
    c_lo = (csc - c_hi.astype(np.float32)).astype(E4)
    planes = np.empty((S, 6, IN, OUT), dtype=E4)
    for q in range(2):
        planes[:, 2 * q] = c_hi[:, q]
        planes[:, 2 * q + 1] = c_lo[:, q]
    planes[:, 4] = c_hi[:, 2]
    planes[:, 5] = c_hi[:, 3]
    # [S, 6, (ich, i), (jt, j)] -> [S, ich(2*icp+k), i, 6, jt, j]
    pl = planes.reshape(S, 6, ICH, 128, JT, 128)
    pl = pl.transpose(0, 2, 3, 1, 4, 5)          # S ich i 6 jt j
    pl = pl.reshape(S, NSG, 2, 128, 6, JT, 128)  # S icp k i 6 jt j
    pl = pl.transpose(1, 0, 3, 2, 4, 5, 6)       # icp S i k 6 jt j
    cr = np.ascontiguousarray(pl.reshape(S * NSG, 128, CT_FREE))

    nc = _build_nc()
    in_maps = [
        {"xt": np.ascontiguousarray(xt[:, g * NB:(g + 1) * NB]),
         "coeff8": cr}
        for g in range(NC)
    ]
    res = run_bass_kernel_spmd(nc, in_maps, core_ids=list(range(NC)),
                               trace=_trace)
    LAST_RESULTS = res
    LAST_EXEC_NS = res.exec_time_ns
    LAST_NC = nc
    LAST_IN_MAPS = in_maps

    yt = np.concatenate([res.results[g]["yt"] for g in range(NC)], axis=1)
    return np.ascontiguousarray(yt.T)


# revision 32
# speedup vs baseline: 2.3526x; 1.0031x over previous
"""Trainium2 Bass kernel for a KAN layer (piecewise-cubic spline edges).

y[b, j] = scale[j] * sum_i sum_p coeff[j, i, seg(x[b,i]), p] * t(x[b,i])^p

with 9 uniform segments on [-1, 1], t the within-segment coordinate.

Strategy (fp8e4m3 DoubleRow matmuls at 0.5 cycles/row):
  * Recast as one-hot-masked GEMM; 8-way data parallel over batch.
  * Coefficients quantized to fp8e4m3 at x64 scale with `scale` folded in.
    For p=0,1 the coeff is split hi/lo and the pair occupies the two
    K-rows of one DoubleRow matmul, with the moving masked-power plane
    fed to both rows via a stride-0 broadcast AP: computes exact
    (c_hi+c_lo)^T @ a at 0.5 cycles/row. For p=2,3 the two K-rows pair
    the two single-fp8 coeff slabs against the (a2, a3) plane pair.
    Measured output rel err 1.75e-2 (vs 2e-2 budget), deterministic.
  * Masked-power planes (fp8) built across DVE (mask, masked-t f32,
    copy), ACT (copy, square), GPSIMD (cube = a1f*t2), each plane
    quantized exactly once from fp32 sources.
  * Chunk-pair supergroups iterate (icp outer, s inner) so chunks 2,3
    setup hides behind the first 9 groups; coeff DMA is one transfer per
    supergroup (serialized DMA is near-critical in the cost model).
"""

import numpy as np
import ml_dtypes

import concourse.bass as bass
import concourse.mybir as mybir
from concourse import bacc
from concourse.tile import TileContext
from concourse.bass_utils import run_bass_kernel_spmd

AF = mybir.ActivationFunctionType
OP = mybir.AluOpType
F32 = mybir.dt.float32
F8 = mybir.dt.float8e4
E4 = ml_dtypes.float8_e4m3
DR = mybir.MatmulPerfMode.DoubleRow

B, IN, OUT = 4096, 512, 512
S, P = 9, 4            # segments, polynomial terms
NC = 8                 # cores
NB = B // NC           # local batch (moving free dim)
ICH = IN // 128        # input chunks (contraction tiles)
JT = OUT // 128        # output-row tiles
NSG = ICH // 2         # chunk-pair supergroups per segment
CSCALE = 64.0          # coeff quantization scale (power of 2)
# free bytes per partition of one supergroup coeff tile:
#   2 chunks * (2 hi/lo pairs * 2 + 2 singles) * JT * 128
CT_FREE = 2 * 6 * JT * 128

AT_BUFS = 4            # in-flight masked-power supergroups
CT_BUFS = 4            # in-flight coeff supergroups
DEBUG_GROUPS = None    # optional [(icp, s), ...] to restrict accumulation

LAST_EXEC_NS = None
LAST_RESULTS = None
LAST_NC = None
LAST_IN_MAPS = None


def _build_nc():
    nc = bacc.Bacc("TRN2", target_bir_lowering=False, debug=False, num_devices=NC)

    xt_d = nc.dram_tensor("xt", [IN, NB], F32, kind="ExternalInput")
    cf_d = nc.dram_tensor("coeff8", [S * NSG, 128, CT_FREE], F8,
                          kind="ExternalInput")
    yt_d = nc.dram_tensor("yt", [OUT, NB], F32, kind="ExternalOutput")

    with TileContext(nc) as tc:
        with (
            tc.tile_pool(name="xp", bufs=1) as xp,
            tc.tile_pool(name="atp", bufs=AT_BUFS) as atp,
            tc.tile_pool(name="ctp", bufs=CT_BUFS) as ctp,
            tc.tile_pool(name="outp", bufs=1) as outp,
            tc.tile_pool(name="pp", bufs=1, space="PSUM") as pp,
        ):
            xt_sb = xp.tile([128, ICH, NB], F32, name="xt_sb")
            u_sb = xp.tile([128, ICH, NB], F32, name="u_sb")
            t_sb = xp.tile([128, ICH, NB], F32, name="t_sb")
            t2_sb = xp.tile([128, ICH, NB], F32, name="t2_sb")
            seg_sb = xp.tile([128, ICH, NB], F32, name="seg_sb")
            segi_sb = xp.tile([128, ICH, NB], mybir.dt.int32, name="segi_sb")
            xt_r = xt_d.rearrange("(c p) b -> p c b", p=128)

            def setup_seg(ic):
                xs = xt_sb[:, ic]
                us = u_sb[:, ic]
                ss = seg_sb[:, ic]
                # u2 = clip(x,-1,1)*4.5 + 4.0 in [-0.5, 8.5]; RNE(u2) == floor
                # of the segment coordinate (verified exact vs searchsorted).
                nc.vector.tensor_scalar(us, xs, 1.0, -1.0, OP.min, OP.max)
                nc.vector.tensor_scalar(us, us, 4.5, 4.0, OP.mult, OP.add)
                nc.vector.tensor_copy(segi_sb[:, ic], us)            # RNE -> int32
                nc.vector.tensor_copy(ss, segi_sb[:, ic])            # back to f32

            def setup_t(ic):
                ts = t_sb[:, ic]
                # t = (u2 + 0.5) - seg
                nc.vector.scalar_tensor_tensor(ts, u_sb[:, ic], 0.5,
                                               seg_sb[:, ic], OP.add,
                                               OP.subtract)
                nc.scalar.activation(t2_sb[:, ic], ts, AF.Square)

            # startup: chunks 0,1 DMA first; setups emitted lazily per chunk
            # so the first group's chunk-0 planes build before chunk 1 setup
            nc.sync.dma_start(xt_sb[:, 0:2], xt_r[:, 0:2])
            setup_done = set()
            t_done = set()

            ps = [pp.tile([128, NB], F32, name=f"ps{jt}", tag=f"ps{jt}")
                  for jt in range(JT)]
            ot = [outp.tile([128, NB], F32, name=f"ot{jt}", tag=f"ot{jt}")
                  for jt in range(JT)]

            groups = [(icp, s) for icp in range(NSG) for s in range(S)]
            if DEBUG_GROUPS is not None:
                groups = [g for g in groups if g in DEBUG_GROUPS]

            def emit_group_tiles(icp, s):
                # coeff supergroup tile: [k, chunk, plane(6), jt, 128]
                # plane layout per chunk: (h0,l0,h1,l1,c2,c3)
                ct = ctp.tile([128, 2, 6, JT, 128], F8,
                              name=f"ct_{icp}_{s}", tag="ct")
                nc.sync.dma_start(
                    ct, cf_d[icp * S + s].rearrange(
                        "p (c q j m) -> p c q j m", c=2, q=6, j=JT))
                a8 = atp.tile([128, 2, P, NB], F8, name=f"a8_{icp}_{s}",
                              tag="a8")
                a1f = atp.tile([128, 2, NB], F32, name=f"a1f_{icp}_{s}",
                               tag="a1f")
                return ct, a8, a1f

            def emit_builders(icp, s, k, a8, a1f):
                ic = 2 * icp + k
                if ic not in setup_done:
                    setup_done.add(ic)
                    setup_seg(ic)
                ss = seg_sb[:, ic]
                nc.vector.tensor_scalar(a8[:, k, 0], ss, float(s),
                                        None, OP.is_equal)
                if ic not in t_done:
                    t_done.add(ic)
                    setup_t(ic)
                nc.vector.scalar_tensor_tensor(a1f[:, k], ss, float(s),
                                               t_sb[:, ic],
                                               OP.is_equal, OP.mult)
                if k % 2:
                    nc.scalar.activation(a8[:, k, 1], a1f[:, k], AF.Copy)
                else:
                    nc.vector.tensor_copy(a8[:, k, 1], a1f[:, k])
                nc.scalar.activation(a8[:, k, 2], a1f[:, k], AF.Square)
                nc.gpsimd.tensor_mul(a8[:, k, 3], a1f[:, k], t2_sb[:, ic])

            def mm(jt, ct, a8, k, q, start=False, stop=False):
                # q0/q1: (hi,lo) pair x broadcast plane;
                # q2: (c2,c3) pair x (a2,a3) planes
                if q < 2:
                    rhs = a8[:, k, q].unsqueeze(1).broadcast_to([128, 2, NB])
                else:
                    rhs = a8[:, k, 2:4]
                nc.tensor.matmul(ps[jt], lhsT=ct[:, k, 2 * q:2 * q + 2, jt],
                                 rhs=rhs, start=start, stop=stop,
                                 perf_mode=DR)

            first = True
            for icp, s in groups[:-2]:
                if icp == 1 and 2 not in setup_done:
                    nc.sync.dma_start(xt_sb[:, 2:4], xt_r[:, 2:4])
                ct, a8, a1f = emit_group_tiles(icp, s)
                for k in range(2):
                    emit_builders(icp, s, k, a8, a1f)
                    for jt in range(JT):
                        for q in range(3):
                            mm(jt, ct, a8, k, q,
                               start=(first and k == 0 and q == 0))
                first = False

            # final two groups: jt-major across both so each PSUM bank
            # stops and drains while later banks still accumulate
            tiles = []
            for icp, s in groups[-2:]:
                ct, a8, a1f = emit_group_tiles(icp, s)
                for k in range(2):
                    emit_builders(icp, s, k, a8, a1f)
                tiles.append((ct, a8))
            inv = 1.0 / CSCALE
            for jt in range(JT):
                for gi, (ct, a8) in enumerate(tiles):
                    for k in range(2):
                        for q in range(3):
                            mm(jt, ct, a8, k, q,
                               stop=(gi == 1 and k == 1 and q == 2))
                if jt % 2:
                    nc.scalar.activation(ot[jt], ps[jt], AF.Copy, scale=inv)
                else:
                    nc.vector.tensor_scalar(ot[jt], ps[jt], inv, None,
                                            OP.mult)
                nc.sync.dma_start(yt_d[jt * 128:(jt + 1) * 128, :], ot[jt])

    nc.compile()
    return nc


def kernel(x, coeff, scale, _trace=False):
    global LAST_EXEC_NS, LAST_RESULTS, LAST_NC, LAST_IN_MAPS
    x = np.ascontiguousarray(np.asarray(x, dtype=np.float32))
    coeff = np.ascontiguousarray(np.asarray(coeff, dtype=np.float32))
    scale = np.ascontiguousarray(np.asarray(scale, dtype=np.float32))

    # x^T shards: [IN, NB] per core
    xt = np.ascontiguousarray(x.T)

    # coeff [OUT, IN, S, P] * scale -> fp8 planes, packed per supergroup as
    # [s*NSG+icp, i_in, (chunk, plane7, jt, j_in)] with plane order
    # (h0,l0,h1,l1,h2,l2,c3)
    csc = coeff.transpose(2, 3, 1, 0) * (scale[None, None, None, :]
                                         * np.float32(CSCALE))  # [S,P,IN,OUT]
    c_hi = csc.astype(E4)
    c_lo = (csc - c_hi.astype(np.float32)).astype(E4)
    planes = np.empty((S, 6, IN, OUT), dtype=E4)
    for q in range(2):
        planes[:, 2 * q] = c_hi[:, q]
        planes[:, 2 * q + 1] = c_lo[:, q]
    planes[:, 4] = c_hi[:, 2]
    planes[:, 5] = c_hi[:, 3]
    # [S, 6, (ich, i), (jt, j)] -> [S, ich(2*icp+k), i, 6, jt, j]
    pl = planes.reshape(S, 6, ICH, 128, JT, 128)
    pl = pl.transpose(0, 2, 3, 1, 4, 5)          # S ich i 6 jt j
    pl = pl.reshape(S, NSG, 2, 128, 6, JT, 128)  # S icp k i 6 jt j
    pl = pl.transpose(1, 0, 3, 2, 4, 5, 6)       # icp S i k 6 jt j
    cr = np.ascontiguousarray(pl.reshape(S * NSG, 128, CT_FREE))

    nc = _build_nc()
    in_maps = [
        {"xt": np.ascontiguousarray(xt[:, g * NB:(g + 1) * NB]),
         "coeff8": cr}
        for g in range(NC)
    ]
    res = run_bass_kernel_spmd(nc, in_maps, core_ids=list(range(NC)),
                               trace=_trace)
    LAST_RESULTS = res
    LAST_EXEC_NS = res.exec_time_ns
    LAST_NC = nc
    LAST_IN_MAPS = in_maps

    yt = np.concatenate([res.results[g]["yt"] for g in range(NC)], axis=1)
    return np.ascontiguousarray(yt.T)
